# revision 10
# baseline (speedup 1.0000x reference)
"""Trainium2 Bass kernel for CausalSelfAttention with block-repeated causal mask.

Problem: B=2, T=3072, C=1024, H=16 heads, d=64.
  q/k/v = x @ W{q,k,v}.T + b;  scores = q k^T / 8, masked by
  (i % 1024) >= (j % 1024) (tril(1024) tiled 3x3), softmax, y = attn @ v,
  out = y @ Wp.T + bp.

Sharding (8 cores): core i handles batch b = i//4 and heads 4*(i%4)..4*(i%4)+3
(data parallel on B, tensor parallel on heads).  Each core computes a partial
output projection (its 4 heads' contribution, no bias); the host sums the 4
partials per batch and adds bp (the TP all-reduce done at unshard time).

Device layout per core (all matmul operands bf16, fp32 accumulation):
  xT    [C=1024, T]   : x[b] transposed (c_in on partitions)
  qT    [128, pair, T]: head-pair-packed [d-channel, t] projections
  kTz   [128, 2*pair, T]: zero-padded per-head kT (K=128 stationary for
        the score matmuls; K=64 row-tiling hangs the device - probed)
  v     [128, ktile, 4*(64+1)] : [t, head*(d | ones)] - ones col => rowsums

Scheduling: an attention query-tile-pair at mask-row r needs q for its own
two tiles plus k/v for rows <= r+1 of ALL 3 blocks.  Projections and the
xT DMA are therefore emitted in row-group WAVES (rows 0-1 first at 256-col
granularity, then rows 2-3, then rows 4-7 at 512), so the first exp hits
the Scalar engine at ~18us instead of ~46us and later waves ride between
attention pairs as PE fill during exp waits.  PSUM (8 banks) carries one
shared [128,1536] double-buffered ring for projection chunks + score
batches (tile-pool rings tie alloc N to release of alloc N-2, so ring
tenancy order IS the schedule), plus a 2-bank ring for the attn@v
accumulators / transpose tiles / output projections.

Attention per query-tile-pair in mask-row batches r=0..ri1: one PSUM tile
[128, 1536] holds scores for (2 heads x 3 blocks x 256q), one big ACT exp
per batch (scale=1/8 folded in, no max-subtraction, |scores| ~ 2);
diagonal masking via ONE [128,6,128] strided tensor_mul per batch; the
r==ri1 batch exps only the live qi=1 halves.  attn@v: lhsT = exp(s^T),
rhs = v_ext -> y psum [q, d|rowsum]; both query tiles' accumulators share
ONE psum bank (elementwise has_written semantics); normalize by
reciprocal rowsum (one strided reciprocal per qi), PE-transpose, output
projection.  Tails are emitted per head-pair so hp0's tail overlaps hp1's
batches; po allocation order (qi1 first, qi0 matmuls emitted first) makes
the next pair wait on the first-completing projection copy.
"""

import numpy as np
import ml_dtypes

import concourse.bass as bass
from concourse import bacc
import concourse.mybir as mybir
from concourse.bass import ts
from concourse.tile import TileContext
from concourse.bass_utils import run_bass_kernel_spmd
from concourse.masks import make_identity, make_upper_triangular

B, T, C, H = 2, 3072, 1024, 16
D = 64                  # head dim
NCORE = 8
HPC = 4                 # heads per core
PAIRS = 2               # head pairs per core
CHS = HPC * D           # 256 channels per core
NKT = T // 128          # 24 key tiles
NQT = T // 128          # 24 query tiles
NQP = NQT // 2          # 12 query tile-pairs
RPB = 8                 # 128-tiles per 1024 mask block
DE = D + 1              # head value cols incl. ones column
BLK = T // 3            # 1024 t-columns per mask block

BF16 = mybir.dt.bfloat16
F32 = mybir.dt.float32

_CACHE = {}


def _build():
    nc = bacc.Bacc()

    xT = nc.dram_tensor("xT", [C, T], BF16, kind="ExternalInput")
    wqT = nc.dram_tensor("wqT", [C, CHS], BF16, kind="ExternalInput")
    wkT = nc.dram_tensor("wkT", [C, CHS], BF16, kind="ExternalInput")
    wvT = nc.dram_tensor("wvT", [C, CHS], BF16, kind="ExternalInput")
    wpT = nc.dram_tensor("wpT", [CHS, C], BF16, kind="ExternalInput")
    bqd = nc.dram_tensor("bq", [128, PAIRS], F32, kind="ExternalInput")
    bkd = nc.dram_tensor("bk", [128, PAIRS], F32, kind="ExternalInput")
    bvd = nc.dram_tensor("bv", [128, CHS], F32, kind="ExternalInput")
    outd = nc.dram_tensor("out", [T, C], F32, kind="ExternalOutput")

    # xT arrives (and q/k projections run) in row-group waves: (t-offset
    # within block, width) pairs; rows 0-1 and 2-3 at 256 cols, rows 4-7
    # at 512.
    WAVES = [(0, 256), (256, 256), (512, 512)]

    with TileContext(nc) as tc:
        with (
            tc.tile_pool(name="const", bufs=1) as const,
            tc.tile_pool(name="qkv", bufs=1) as qkvp,
            tc.tile_pool(name="exps", bufs=8) as expp,
            tc.tile_pool(name="ynorm", bufs=8) as ynp,
            tc.tile_pool(name="ytp", bufs=6) as ytp,
            tc.tile_pool(name="outp", bufs=3) as outp,
            tc.tile_pool(name="small", bufs=16) as smallp,
            tc.tile_pool(name="ps_s", bufs=2, space="PSUM") as ps_s,
            tc.tile_pool(name="ps_a", bufs=2, space="PSUM") as ps_a,
        ):
            # ---------------- constants / weights into SBUF ----------------
            wq_sb = const.tile([128, C // 128, CHS], BF16)
            wk_sb = const.tile([128, C // 128, CHS], BF16)
            wv_sb = const.tile([128, C // 128, CHS], BF16)
            wp_sb = const.tile([128, PAIRS, C], BF16)
            bq_ld = const.tile([128, PAIRS], F32)
            bk_ld = const.tile([128, PAIRS], F32)
            bv_ld = const.tile([128, CHS], F32)
            xT_sb = const.tile([128, C // 128, T], BF16)
            xT_ap = xT[:, :].rearrange("(a p) t -> p a t", p=128)

            nc.sync.dma_start(
                out=wq_sb, in_=wqT[:, :].rearrange("(a p) c -> p a c", p=128)
            )
            nc.scalar.dma_start(
                out=wk_sb, in_=wkT[:, :].rearrange("(a p) c -> p a c", p=128)
            )
            nc.sync.dma_start(
                out=wv_sb, in_=wvT[:, :].rearrange("(a p) c -> p a c", p=128)
            )
            nc.scalar.dma_start(out=bq_ld, in_=bqd[:, :])
            nc.scalar.dma_start(out=bk_ld, in_=bkd[:, :])
            nc.scalar.dma_start(out=bv_ld, in_=bvd[:, :])
            # xT lands in wave order (rows 0-1 of all blocks first), each
            # chunk matching a projection matmul's read range exactly so
            # the first scores run ~18us in
            n = 0
            for wi, (t0, w) in enumerate(WAVES):
                for b in range(3):
                    for ci in range(C // 128):
                        eng = nc.sync if n % 2 == 0 else nc.scalar
                        eng.dma_start(
                            out=xT_sb[:, ci, b * BLK + t0 : b * BLK + t0 + w],
                            in_=xT_ap[:, ci, b * BLK + t0 : b * BLK + t0 + w],
                        )
                        n += 1
                if wi == 0:
                    nc.scalar.dma_start(
                        out=wp_sb, in_=wpT[:, :].rearrange("(a p) c -> p a c", p=128)
                    )
            # DVE-local copies: consumers then never need a DMA sem wait
            # (walrus allows only one sync-wait on TensorScalar/TensorTensor)
            bq_sb = const.tile([128, PAIRS], F32)
            bk_sb = const.tile([128, PAIRS], F32)
            bv_sb = const.tile([128, CHS], F32)
            nc.vector.tensor_copy(bq_sb, bq_ld)
            nc.vector.tensor_copy(bk_sb, bk_ld)
            nc.vector.tensor_copy(bv_sb, bv_ld)

            ident = const.tile([128, 128], BF16)
            make_identity(nc, ident)
            # mask[k', q'] = 1 where q' >= k' (keep), else 0
            mask_sb = const.tile([128, 128], BF16)
            make_upper_triangular(nc, mask_sb, val=1.0, diag=True)
            mask_bc6 = mask_sb.unsqueeze(1).to_broadcast((128, 6, 128))

            qT_sb = qkvp.tile([128, PAIRS, T], BF16)
            kTz = qkvp.tile([128, PAIRS * 2, T], BF16)
            nc.gpsimd.memset(kTz, 0.0)
            v_sb = qkvp.tile([128, NKT, HPC * DE], BF16)
            nc.vector.memset(v_sb, 1.0)  # ones columns for rowsums

            # ---------------- projection helpers ----------------
            def emit_qk(b, t0, w):
                # q/k for t-range [b*BLK+t0, +w): psum cols [0:w] hold q,
                # [512:512+w] hold k (fixed offsets inside the 1536 slot)
                lo = b * BLK + t0
                for pr in range(PAIRS):
                    pq = ps_s.tile([128, 1536], F32, name="pq", tag="sc")
                    for ci in range(C // 128):
                        nc.tensor.matmul(
                            pq[:, 0:w],
                            lhsT=wq_sb[:, ci, ts(pr, 128)],
                            rhs=xT_sb[:, ci, lo : lo + w],
                            start=(ci == 0),
                            stop=(ci == C // 128 - 1),
                        )
                    for ci in range(C // 128):
                        nc.tensor.matmul(
                            pq[:, 512 : 512 + w],
                            lhsT=wk_sb[:, ci, ts(pr, 128)],
                            rhs=xT_sb[:, ci, lo : lo + w],
                            start=(ci == 0),
                            stop=(ci == C // 128 - 1),
                        )
                    nc.vector.tensor_add(
                        qT_sb[:, pr, lo : lo + w],
                        pq[:, 0:w],
                        bq_sb[:, pr : pr + 1].to_broadcast((128, w)),
                    )
                    nc.vector.tensor_add(
                        kTz[0:D, pr * 2, lo : lo + w],
                        pq[0:D, 512 : 512 + w],
                        bk_sb[0:D, pr : pr + 1].to_broadcast((D, w)),
                    )
                    nc.vector.tensor_add(
                        kTz[D:128, pr * 2 + 1, lo : lo + w],
                        pq[D:128, 512 : 512 + w],
                        bk_sb[D:128, pr : pr + 1].to_broadcast((D, w)),
                    )

            bv_r = bv_sb.rearrange("p (h e) -> p h e", e=D)

            def emit_v(g):
                # v for mask-rows {2g, 2g+1} of all 3 blocks (6 key tiles)
                tiles = [b * RPB + 2 * g + j for b in range(3) for j in range(2)]
                pv = ps_s.tile([128, 1536], F32, name="pv", tag="sc")
                for sub, tt in enumerate(tiles):
                    for ci in range(C // 128):
                        nc.tensor.matmul(
                            pv[:, ts(sub, 256)],
                            lhsT=xT_sb[:, ci, ts(tt, 128)],
                            rhs=wv_sb[:, ci, :],
                            start=(ci == 0),
                            stop=(ci == C // 128 - 1),
                        )
                for sub, tt in enumerate(tiles):
                    vt = v_sb[:, tt, :].rearrange("p (h e) -> p h e", e=DE)[:, :, 0:D]
                    pvr = pv[:, ts(sub, 256)].rearrange("p (h e) -> p h e", e=D)
                    nc.vector.tensor_add(vt, pvr, bv_r)

            # ---------------- attention + output projection ----------------
            def emit_attention(qp):
                ri0 = (2 * qp) % RPB
                ri1 = ri0 + 1
                q0 = 2 * qp

                yns = []
                yts = []
                for hp in range(PAIRS):
                    # py[qi] share one psum bank: only the very first MM
                    # uses start=True (elementwise has_written semantics
                    # let later regions overwrite via cleared bits)
                    py = ps_a.tile([128, 2, 2 * DE], F32, name="py", tag="a")
                    pyq = [py[:, 0, :], py[:, 1, :]]
                    for r in range(ri1 + 1):
                        psc = ps_s.tile([128, 1536], F32, name="psc", tag="sc")
                        # col(h, b, qi) = h*768 + b*256 + qi*128
                        # at r == ri1 the qi=0 half is fully masked: emit
                        # N=128 matmuls covering only the live qi=1 columns
                        qoff = 128 if r == ri1 else 0
                        for b in range(3):
                            J = b * RPB + r
                            for h in range(2):
                                dst = psc[
                                    :,
                                    h * 768 + b * 256 + qoff : h * 768 + b * 256 + 256,
                                ]
                                nc.tensor.matmul(
                                    dst,
                                    lhsT=kTz[:, hp * 2 + h, ts(J, 128)],
                                    rhs=qT_sb[
                                        :, hp, q0 * 128 + qoff : q0 * 128 + 256
                                    ],
                                    start=True,
                                    stop=True,
                                )
                        esb = expp.tile([128, 1536], BF16)
                        e6 = esb.rearrange("p (x c) -> p x c", c=256)
                        p6 = psc.rearrange("p (x c) -> p x c", c=256)
                        if r == ri1:
                            # qi=0 halves are fully masked: skip their exp
                            nc.scalar.activation(
                                e6[:, :, 128:256], p6[:, :, 128:256],
                                mybir.ActivationFunctionType.Exp, scale=0.125,
                            )
                        else:
                            nc.scalar.activation(
                                esb, psc,
                                mybir.ActivationFunctionType.Exp, scale=0.125,
                            )
                        if r in (ri0, ri1):
                            # diagonal tiles: zero the masked upper triangle
                            # in ONE strided op over all 6 (h,b) groups
                            qi = 0 if r == ri0 else 1
                            sl = e6[:, :, qi * 128 : qi * 128 + 128]
                            nc.vector.tensor_mul(sl, sl, mask_bc6)
                        # attn @ v_ext -> y psum [q, d|rowsum] accumulation
                        for b in range(3):
                            J = b * RPB + r
                            for h in range(2):
                                hg = hp * 2 + h
                                for qi in range(2):
                                    if qi == 0 and r == ri1:
                                        continue
                                    st = r == 0 and b == 0 and h == 0 and qi == 0
                                    nc.tensor.matmul(
                                        pyq[qi][:, h * DE : (h + 1) * DE],
                                        lhsT=esb[
                                            :,
                                            h * 768 + b * 256 + qi * 128 : h * 768 + b * 256 + qi * 128 + 128,
                                        ],
                                        rhs=v_sb[:, J, hg * DE : (hg + 1) * DE],
                                        start=st,
                                        stop=(b == 2 and r == (ri0 if qi == 0 else ri1)),
                                        skip_group_check=True,
                                    )
                    # per-pair tail: normalize + transpose emitted inside the
                    # hp loop so hp0's tail overlaps hp1's score/exp batches.
                    # One strided reciprocal covers both heads' rowsum cols.
                    py3 = py.rearrange("p q (h e) -> p q h e", e=DE)
                    for qi in range(2):
                        rc = smallp.tile([128, 2], F32)
                        nc.vector.reciprocal(rc, py3[:, qi, :, D])
                        yn = ynp.tile([128, 128], BF16)
                        for h in range(2):
                            nc.vector.tensor_scalar_mul(
                                yn[:, h * D : (h + 1) * D],
                                pyq[qi][:, h * DE : h * DE + D],
                                rc[:, h : h + 1],
                            )
                        yns.append(yn)
                    pyt = ps_a.tile([128, 2, 128], BF16, name="pyt", tag="a")
                    for qi in range(2):
                        nc.tensor.transpose(pyt[:, qi, :], yns[2 * hp + qi], ident)
                    yt = ytp.tile([128, 2, 128], BF16)
                    nc.vector.tensor_copy(yt, pyt)
                    yts.append(yt)

                # output projection for the two query tiles.  po allocation
                # order (qi1 first) puts qi0's tiles in the ring positions
                # that gate the NEXT pair's py allocs, and qi0's matmuls and
                # copies are emitted first, so the next pair's attn@v waits
                # on the FIRST-completing projection copy, not the last.
                pos = {}
                for qi in (1, 0):
                    for ch in range(2):
                        pos[(qi, ch)] = ps_a.tile([128, 512], F32, name="po", tag="a")
                for qi in (0, 1):
                    qt = q0 + qi
                    osb = outp.tile([128, C], F32)
                    for ch in range(2):
                        po = pos[(qi, ch)]
                        for hp in range(PAIRS):
                            nc.tensor.matmul(
                                po,
                                lhsT=yts[hp][:, qi, :],
                                rhs=wp_sb[:, hp, ts(ch, 512)],
                                start=(hp == 0),
                                stop=(hp == PAIRS - 1),
                            )
                        nc.vector.tensor_copy(osb[:, ts(ch, 512)], po)
                        nc.sync.dma_start(
                            out=outd[qt * 128 : (qt + 1) * 128, ts(ch, 512)],
                            in_=osb[:, ts(ch, 512)],
                        )

            # ---------------- emission schedule ----------------
            # wave 0: rows 0-1 -> the three row-0/1 pairs; wave 1: rows 2-3;
            # wave 2: rows 4-7 (+ v rows 4-5, then 6-7).  Later waves ride
            # between attention pairs as PE fill during exp waits; heavy
            # pairs (8 exp batches) start as early as deps allow; the final
            # pair is light so the drain tail is short.
            for b in range(3):
                emit_qk(b, 0, 256)
            emit_v(0)
            emit_attention(0)
            emit_attention(4)
            for b in range(3):
                emit_qk(b, 256, 256)
            emit_attention(8)
            emit_v(1)
            emit_attention(1)
            emit_qk(0, 512, 512)
            emit_attention(5)
            emit_qk(1, 512, 512)
            emit_attention(9)
            emit_qk(2, 512, 512)
            emit_v(2)
            emit_attention(2)
            emit_v(3)
            emit_attention(3)
            emit_attention(7)
            emit_attention(6)
            emit_attention(11)
            emit_attention(10)

    nc.finalize()  # Bacc: runs compile pipeline (event-sem split, reg alloc)
    return nc


def _get_nc():
    if "nc" not in _CACHE:
        _CACHE["nc"] = _build()
    return _CACHE["nc"]


def _shard(inputs):
    bf = ml_dtypes.bfloat16
    x = np.asarray(inputs["x"], dtype=np.float32)
    Wq = np.asarray(inputs["Wq"], dtype=np.float32)
    Wk = np.asarray(inputs["Wk"], dtype=np.float32)
    Wv = np.asarray(inputs["Wv"], dtype=np.float32)
    Wp = np.asarray(inputs["Wp"], dtype=np.float32)
    bq = np.asarray(inputs["bq"], dtype=np.float32)
    bk = np.asarray(inputs["bk"], dtype=np.float32)
    bv = np.asarray(inputs["bv"], dtype=np.float32)

    in_maps = []
    for i in range(NCORE):
        b = i // 4
        j = i % 4
        hs = slice(j * CHS, (j + 1) * CHS)
        m = {
            "xT": np.ascontiguousarray(x[b].T).astype(bf),
            "wqT": np.ascontiguousarray(Wq[hs].T).astype(bf),
            "wkT": np.ascontiguousarray(Wk[hs].T).astype(bf),
            "wvT": np.ascontiguousarray(Wv[hs].T).astype(bf),
            "wpT": np.ascontiguousarray(Wp[:, hs].T).astype(bf),
            "bq": np.ascontiguousarray(bq[hs].reshape(PAIRS, 128).T),
            "bk": np.ascontiguousarray(bk[hs].reshape(PAIRS, 128).T),
            "bv": np.ascontiguousarray(np.broadcast_to(bv[hs], (128, CHS))),
        }
        in_maps.append(m)
    return in_maps


def _unshard(results, inputs):
    bp = np.asarray(inputs["bp"], dtype=np.float32)
    out = np.empty((B, T, C), dtype=np.float32)
    for b in range(B):
        acc = results[4 * b]["out"].astype(np.float32).copy()
        for j in range(1, 4):
            acc += results[4 * b + j]["out"]
        out[b] = acc + bp
    return out


def run(inputs, trace=False):
    nc = _get_nc()
    in_maps = _shard(inputs)
    res = run_bass_kernel_spmd(nc, in_maps, list(range(NCORE)), trace=trace)
    return _unshard(res.results, inputs), res


def kernel(**inputs):
    out, _ = run(inputs, trace=False)
    return out
